# revision 1
# baseline (speedup 1.0000x reference)
"""GQA (16 Q heads / 4 KV heads, causal) for Trainium2, 8 NeuronCores.

Sharding: core = b*4 + j  (b = batch 0..1, j = KV-head group 0..3).
Each core computes attention for its batch b and its 4 Q heads (KV head j)
over the full 2048-token sequence, projects through its Wo row-slice, and a
ReduceScatter(add) over the 4 cores of each batch sums the Wo partials and
leaves each core with a 512-token slice of the final output.

Everything on-chip runs in "transposed activation" space (features on
partitions, tokens on the free dim) so no transposes are needed anywhere
except a cheap 64x2048 PE transpose for V:
  qT = WqT.T @ xT   (Wq column-slice as the stationary operand, pre-scaled
                     by 1/sqrt(64) on host)
  scoresT[s, n] = (kT tile).T @ qT           -> softmax runs over the
                     partition (s) axis: exp on ACT, the denominator comes
                     free out of the PV matmul via a ones-column in V, and
                     the division folds into a [64, 512] multiply.
  outT[d, n] = v[s, d+ones].T @ expT[s, n]   (accumulated over s tiles)
  yT partial = Wo row-slice.T @ attnoutT     -> y in natural layout
Causality: s-tiles strictly above the diagonal are skipped entirely (the
matmuls only cover the valid n-range) and diagonal 128x128 blocks are masked
with one multiply by a host-provided triangular 0/1 mask, post-exp.
Matmuls use float32r (TF32-like) which runs at full PE rate for free dim
>= 256; accumulation stays fp32.
"""

import sys

sys.path.insert(0, "/opt/trn_rl_repo")

import numpy as np

import concourse.bass as bass
import concourse.mybir as mybir
import concourse.tile as tile
from concourse import bacc
from concourse.bass_utils import run_bass_kernel_spmd

F32 = mybir.dt.float32
F32R = mybir.dt.float32r
EXP = mybir.ActivationFunctionType.Exp
MULT = mybir.AluOpType.mult
ADD = mybir.AluOpType.add

B, N, E = 2, 2048, 1024
D = 64          # head dim
HL = 4          # heads per core
KT = 8          # 1024 channels / 128
NCH = 4         # n chunks of 512
NT = 16         # token tiles of 128
GROUPS = [[0, 1, 2, 3], [4, 5, 6, 7]]

_NC_CACHE = {}


def _r(ap):
    return ap.bitcast(F32R)


def build_program():
    nc = bacc.Bacc("TRN2", target_bir_lowering=False, debug=False)
    nc.num_devices = 8

    xT_d = nc.dram_tensor("xT", [E, N], F32R, kind="ExternalInput")
    wqkv_d = nc.dram_tensor("wqkv", [E, 384], F32R, kind="ExternalInput")
    wo_d = nc.dram_tensor("wo", [256, E], F32R, kind="ExternalInput")
    mask_d = nc.dram_tensor("trimask", [128, 128], F32, kind="ExternalInput")
    ident_d = nc.dram_tensor("ident", [64, 64], F32R, kind="ExternalInput")
    out_d = nc.dram_tensor("y_rs", [N // 4, E], F32, kind="ExternalOutput")

    with tile.TileContext(nc) as tc:
        with (
            tc.tile_pool(name="const", bufs=1) as cpool,
            tc.tile_pool(name="qkv", bufs=1) as qpool,
            tc.tile_pool(name="work", bufs=6) as wpool,
            tc.tile_pool(name="small", bufs=4) as spool,
            tc.tile_pool(name="dram", bufs=1, space="DRAM") as dpool,
        ):
            xT_sb = cpool.tile([128, KT, N], F32R)
            wqkv_sb = cpool.tile([128, KT, 384], F32R)
            wo_sb = cpool.tile([128, 2, E], F32R)
            mask_sb = cpool.tile([128, 128], F32)
            ident_sb = cpool.tile([64, 64], F32R)
            ones_sb = cpool.tile([1, 64], F32R)
            nc.vector.memset(ones_sb[:].bitcast(F32), 1.0)

            for k in range(KT):
                nc.sync.dma_start(wqkv_sb[:, k, :], wqkv_d[k * 128 : (k + 1) * 128, :])
            for ch in range(NCH):
                for k in range(KT):
                    nc.sync.dma_start(
                        xT_sb[:, k, ch * 512 : (ch + 1) * 512],
                        xT_d[k * 128 : (k + 1) * 128, ch * 512 : (ch + 1) * 512],
                    )
            for k in range(2):
                nc.sync.dma_start(wo_sb[:, k, :], wo_d[k * 128 : (k + 1) * 128, :])
            nc.sync.dma_start(mask_sb[:], mask_d[:])
            nc.sync.dma_start(ident_sb[:], ident_d[:])

            qT_sb = qpool.tile([64, HL, N], F32R)    # [:, h, :] = head h, base part 0
            kT_sb = qpool.tile([64, N], F32R)
            vT_sb = qpool.tile([64, N], F32R)
            vnat_sb = qpool.tile([128, NT, 66], F32R)  # [:, t, 0:64]=v, [:, t, 64]=1
            attnoutT_c = [
                qpool.tile([128, 2, 512], F32R, name=f"attnoutT{c}", tag=f"attnoutT{c}")
                for c in range(NCH)
            ]

            # ---- projections: stacked [Wq | Wk | Wv] lhsT, xT rhs ----
            with (
                tc.tile_pool(name="proj_ps", bufs=2, space="PSUM") as proj_ps,
                tc.tile_pool(name="tr_ps", bufs=2, space="PSUM") as tr_ps,
            ):
                nc.vector.memset(vnat_sb[:, :, 64:65].bitcast(F32), 1.0)
                for ch in range(NCH):
                    sl = slice(ch * 512, (ch + 1) * 512)
                    for m in range(3):
                        ps = proj_ps.tile([128, 512], F32)
                        for k in range(KT):
                            nc.tensor.matmul(
                                ps[:],
                                (wqkv_sb[:, k, m * 128 : (m + 1) * 128]),
                                (xT_sb[:, k, ch * 512 : (ch + 1) * 512]),
                                start=(k == 0),
                                stop=(k == KT - 1),
                            )
                        if m < 2:
                            nc.scalar.copy(qT_sb[:, 2 * m, sl], ps[0:64, :])
                            nc.scalar.copy(qT_sb[:, 2 * m + 1, sl], ps[64:128, :])
                        else:
                            nc.scalar.copy(kT_sb[:, sl], ps[0:64, :])
                            nc.scalar.copy(vT_sb[:, sl], ps[64:128, :])
                    for t in range(4 * ch, 4 * ch + 4):
                        tp = tr_ps.tile([128, 64], F32R)
                        nc.tensor.transpose(
                            tp[:], vT_sb[:, t * 128 : (t + 1) * 128], ident_sb[:]
                        )
                        nc.vector.tensor_copy(vnat_sb[:, t, 0:64], tp[:])

            y_parts = [
                dpool.tile([512, E], F32, name=f"y_part{c}", tag=f"y_part{c}")
                for c in range(NCH)
            ]
            y_rss = [
                dpool.tile([128, E], F32, name=f"y_rs{c}", tag=f"y_rs{c}")
                for c in range(NCH)
            ]

            with (
                tc.tile_pool(name="qk_ps", bufs=3, space="PSUM") as qk_ps,
                tc.tile_pool(name="pv_ps", bufs=2, space="PSUM") as pv_ps,
                tc.tile_pool(name="y_ps", bufs=2, space="PSUM") as y_ps,
                tc.tile_pool(name="bc_ps", bufs=1, space="PSUM") as bc_ps,
            ):
                for c in range(NCH):
                    nsl = slice(c * 512, (c + 1) * 512)
                    nst = 4 * c + 4
                    for h in range(HL):
                        pv = pv_ps.tile([65, 512], F32)
                        for i in range(nst):
                            f0 = max(0, 128 * (i - 4 * c))
                            qk = qk_ps.tile([128, 512], F32)
                            nc.tensor.matmul(
                                qk[:, f0:512],
                                (kT_sb[:, i * 128 : (i + 1) * 128]),
                                (qT_sb[:, h, c * 512 + f0 : (c + 1) * 512]),
                                start=True,
                                stop=True,
                            )
                            if i >= 4 * c:
                                nc.vector.scalar_tensor_tensor(
                                    out=qk[:, f0 : f0 + 128],
                                    in0=qk[:, f0 : f0 + 128],
                                    scalar=1.0,
                                    in1=mask_sb[:],
                                    op0=MULT,
                                    op1=ADD,
                                )
                            pT = wpool.tile([128, 512], F32R, tag="pT")
                            nc.scalar.activation(pT[:, f0:512], qk[:, f0:512], EXP)
                            nc.tensor.matmul(
                                pv[:, f0:512],
                                (vnat_sb[:, i, 0:65]),
                                (pT[:, f0:512]),
                                start=(i == 0),
                                stop=(i == nst - 1),
                            )
                        recip = spool.tile([1, 512], F32R, tag="recip")
                        bcast = spool.tile([64, 512], F32, tag="bcast")
                        with nc.allow_low_precision(reason="recip feeds f32r bcast matmul"):
                            nc.vector.reciprocal(recip[:], pv[64:65, :])
                        bc = bc_ps.tile([64, 512], F32)
                        nc.tensor.matmul(bc[:], ones_sb[:], recip[:], start=True, stop=True)
                        nc.vector.tensor_copy(bcast[:], bc[:])
                        arow = (h % 2) * 64
                        ablk = h // 2
                        nc.vector.scalar_tensor_tensor(
                            out=attnoutT_c[c][arow : arow + 64, ablk, :],
                            in0=pv[0:64, :],
                            scalar=1.0,
                            in1=bcast[:],
                            op0=MULT,
                            op1=MULT,
                        )
                    # Wo projection for this chunk's tokens
                    for t in range(4 * c, 4 * c + 4):
                        for e in range(2):
                            yp = y_ps.tile([128, 512], F32)
                            for kb in range(2):
                                nc.tensor.matmul(
                                    yp[:],
                                    (attnoutT_c[c][:, kb, (t - 4 * c) * 128 : (t - 4 * c + 1) * 128]),
                                    (wo_sb[:, kb, e * 512 : (e + 1) * 512]),
                                    start=(kb == 0),
                                    stop=(kb == 1),
                                )
                            y_sb = wpool.tile([128, 512], F32, tag="y_sb")
                            nc.vector.tensor_copy(y_sb[:], yp[:])
                            tl = t - 4 * c
                            tgt = y_parts[c][tl * 128 : (tl + 1) * 128, e * 512 : (e + 1) * 512]
                            nc.sync.dma_start(tgt, y_sb[:])
                    nc.gpsimd.collective_compute(
                        "ReduceScatter",
                        mybir.AluOpType.add,
                        replica_groups=GROUPS,
                        ins=[y_parts[c].opt()],
                        outs=[y_rss[c].opt()],
                    )
                    nc.gpsimd.dma_start(out_d[c * 128 : (c + 1) * 128, :], y_rss[c][:])

    nc.finalize()
    return nc


def get_program():
    if "nc" not in _NC_CACHE:
        _NC_CACHE["nc"] = build_program()
    return _NC_CACHE["nc"]


def make_in_maps(x, Wq, Wk, Wv, Wo):
    trimask = np.where(
        np.arange(128)[:, None] <= np.arange(128)[None, :], 0.0, -1e30
    ).astype(np.float32)
    ident = np.eye(64, dtype=np.float32)
    xT = [np.ascontiguousarray(x[b].T).astype(np.float32) for b in range(B)]
    in_maps = []
    for core in range(8):
        b, j = core // 4, core % 4
        wqkv = np.ascontiguousarray(
            np.concatenate(
                [
                    Wq[:, j * 256 : (j + 1) * 256] * (1.0 / np.sqrt(D)),
                    Wk[:, j * 64 : (j + 1) * 64],
                    Wv[:, j * 64 : (j + 1) * 64],
                ],
                axis=1,
            )
        ).astype(np.float32)
        wo = np.ascontiguousarray(Wo[j * 256 : (j + 1) * 256, :]).astype(np.float32)
        in_maps.append({"xT": xT[b], "wqkv": wqkv, "wo": wo, "trimask": trimask, "ident": ident})
    return in_maps


def gather_output(results):
    y = np.empty((B, N, E), dtype=np.float32)
    for core in range(8):
        b, j = core // 4, core % 4
        piece = results[core]["y_rs"]
        for c in range(NCH):
            r0 = 512 * c + 128 * j
            y[b, r0 : r0 + 128, :] = piece[128 * c : 128 * c + 128]
    return y


def kernel(x, Wq, Wk, Wv, Wo, _trace=False, **trace_kwargs):
    nc = get_program()
    in_maps = make_in_maps(
        np.asarray(x), np.asarray(Wq), np.asarray(Wk), np.asarray(Wv), np.asarray(Wo)
    )
    res = run_bass_kernel_spmd(nc, in_maps, list(range(8)), trace=_trace, **trace_kwargs)
    out = gather_output(res.results)
    if _trace:
        return out, res
    return out

